# revision 4
# baseline (speedup 1.0000x reference)
"""Trainium2 Bass kernel for nn_AverageAttention (B=8, L=2048, D=1024).

Math (per batch b):
    avg[t]  = cumsum(x, axis=t)[t] / (t+1)
    g       = concat([x, avg], -1) @ W_gate.T + b_gate        # (L, 2*D)
    out     = sigmoid(g[:, :D]) * x + sigmoid(g[:, D:]) * avg

Strategy: batch-parallel over 8 NeuronCores (one sequence per core), W_gate
replicated. On-chip layout is transposed (feature-on-partition,
token-on-free) so the cumulative sum is one DVE tensor_tensor_scan per
128-feature chunk. I/O is bf16 (x in, avg/gating out; fp32 upcast on host).

The gating matmul runs, by default, in fp8e4 with DoubleRow perf mode
(2 contraction rows per PE cell per cycle, ~2x bf16 throughput), fp32 PSUM
accumulation. Scale scheme keeps every fp8 operand in the e4m3 normal range
(TRN e4m3 max is +-240):
    x-half:   cat = x,       W' = W_x * 32
    avg-half: cat = avg * 8, W' = W_a * 4
so every partial product carries a 32x scale, undone by the activation
scale (sigmoid(psum/32 + bias)) during PSUM evacuation. Measured on the
reference inputs this lands at rel_l2 ~1.2e-2 (tolerance 2e-2); the bf16
matmul path (KMM=bf16) is ~2x slower on the PE but ~6x more accurate.
"""

from contextlib import ExitStack

import ml_dtypes
import numpy as np

import concourse.bass as bass
import concourse.bass_utils as bass_utils
import concourse.mybir as mybir
import concourse.tile as tile
from concourse import bacc
from concourse._compat import with_exitstack
from concourse.bass import ts

B, L, D = 8, 2048, 1024
NJ = D // 128         # 8 feature chunks (x/avg pairs)
NOB = 2 * D // 128    # 16 output-feature blocks of g
import os as _os_mod

TCW = int(_os_mod.environ.get("KTCW", "512"))  # matmul moving free-dim
NTC = L // TCW

FP32 = mybir.dt.float32
BF16 = mybir.dt.bfloat16
FP8 = mybir.dt.float8e4

# fp8 scale scheme (see module docstring)
SXW, SAW, SACAT = 32.0, 4.0, 8.0
ACT_SCALE = 1.0 / 32.0


@with_exitstack
def _tile_body(
    ctx: ExitStack,
    tc: tile.TileContext,
    mm: str = "fp8",
    reps: int = 1,
):
    nc = tc.nc
    fp8 = mm == "fp8"
    wdt = FP8 if fp8 else BF16

    xTb = nc.dram_tensor("xTb", (NJ, 128, L), BF16, kind="ExternalInput").ap()
    wob = nc.dram_tensor("wob", (NOB, 128, NJ, 2, 128), wdt, kind="ExternalInput").ap()
    invd = nc.dram_tensor("invd", (128, L), FP32, kind="ExternalInput").ap()
    biash = nc.dram_tensor("biash", (128, NOB), FP32, kind="ExternalInput").ap()
    avgT = nc.dram_tensor("avgT", (NJ, 128, L), BF16, kind="ExternalOutput").ap()
    gatT = nc.dram_tensor("gatT", (NJ, 128, L), BF16, kind="ExternalOutput").ap()

    cat_pool = ctx.enter_context(tc.tile_pool(name="cat", bufs=NJ))
    const_pool = ctx.enter_context(tc.tile_pool(name="const", bufs=1))
    cum_pool = ctx.enter_context(tc.tile_pool(name="cum", bufs=2))
    w_pool = ctx.enter_context(tc.tile_pool(name="w", bufs=3))
    sig_pool = ctx.enter_context(tc.tile_pool(name="sig", bufs=3))
    tmp_pool = ctx.enter_context(tc.tile_pool(name="tmp", bufs=2))
    gat_pool = ctx.enter_context(tc.tile_pool(name="gat", bufs=2))
    psum_pool = ctx.enter_context(
        tc.tile_pool(name="psum", bufs=max(1, 8 * 512 // TCW), space="PSUM")
    )
    if fp8:
        x_pool = ctx.enter_context(tc.tile_pool(name="x", bufs=NJ))
        avg_pool = ctx.enter_context(tc.tile_pool(name="avg", bufs=NJ))

    invd_sb = const_pool.tile([128, L], BF16, tag="invd")
    bias_sb = const_pool.tile([128, NOB], FP32, tag="bias")
    if fp8:
        invd8_sb = const_pool.tile([128, L], BF16, tag="invd8")

    # cat[j]: [128, 2, L]; [:, 0, :] = x chunk j, [:, 1, :] = avg chunk j
    # (avg scaled by SACAT on the fp8 path).
    cats = [
        cat_pool.tile([128, 2, L], wdt, tag="cat", name=f"cat{j}") for j in range(NJ)
    ]

    def load_pair_w(j, chunks=1):
        # Pair j covers output blocks j (input gate) and NJ+j (forget gate).
        wt_i = w_pool.tile([128, NJ, 2, 128], wdt, name="wt_i", tag="wt_i")
        wt_f = w_pool.tile([128, NJ, 2, 128], wdt, name="wt_f", tag="wt_f")
        step = NJ // chunks
        for c in range(chunks):
            cs = slice(c * step, (c + 1) * step)
            nc.sync.dma_start(wt_i[:, cs, :, :], wob[j][:, cs, :, :])
            nc.sync.dma_start(wt_f[:, cs, :, :], wob[NJ + j][:, cs, :, :])
        return wt_i, wt_f

    for _rep in range(reps):
        # Head ordering on the sync HWDGE ring (FIFO per ring): first x
        # chunk, then the first pair's W in chunks, so the first matmul
        # unblocks as early as possible. Constants ride the scalar-engine
        # HWDGE ring so they never queue ahead of inputs.
        if fp8:
            xts = [x_pool.tile([128, L], BF16, name="xt", tag="xt") for _ in range(NJ)]
            avbs = [avg_pool.tile([128, L], BF16, name="av", tag="av") for _ in range(NJ)]
            x_dsts = [xt[:] for xt in xts]
        else:
            x_dsts = [cats[j][:, 0, :] for j in range(NJ)]
        nc.sync.dma_start(x_dsts[0], xTb[0])
        if _rep == 0:
            nc.scalar.dma_start(bias_sb[:], biash[:])
            nc.scalar.dma_start(invd_sb[:], invd[:])
            if fp8:
                nc.vector.tensor_scalar_mul(invd8_sb[:], invd_sb[:], SACAT)
        w_tiles = {j: load_pair_w(j, chunks=4 if j == 0 else 1) for j in range(2)}

        # Phase 1: per feature chunk j — load x (bf16), full-width cumsum
        # scan on the DVE (fp32 state), scale by 1/(t+1) on Pool, store avg.
        # fp8 path additionally casts x (ACT) and avg*8 (Pool) into cat.
        for j in range(NJ):
            if j > 0:
                nc.sync.dma_start(x_dsts[j], xTb[j])
            ct = cum_pool.tile([128, L], FP32)
            nc.vector.tensor_tensor_scan(
                ct[:],
                x_dsts[j],
                x_dsts[j],
                0.0,
                mybir.AluOpType.add,
                mybir.AluOpType.bypass,
            )
            if fp8:
                nc.scalar.copy(cats[j][:, 0, :], xts[j][:])
                nc.gpsimd.tensor_mul(avbs[j][:], ct[:], invd_sb[:])
                nc.gpsimd.tensor_mul(cats[j][:, 1, :], ct[:], invd8_sb[:])
                nc.sync.dma_start(avgT[j], avbs[j][:])
            else:
                nc.gpsimd.tensor_mul(cats[j][:, 1, :], ct[:], invd_sb[:])
                nc.sync.dma_start(avgT[j], cats[j][:, 1, :])

        # Phase 2: gating matmul g^T = W @ cat^T per 128-row output block in
        # weight-stationary order (the NTC token chunks run as interleaved
        # PSUM groups so consecutive matmuls share stationary weights);
        # sigmoid(psum * ACT_SCALE + bias) fused into PSUM evacuation on the
        # scalar engine; elementwise gate combine per pair.
        if fp8:
            x_src = [xts[j][:] for j in range(NJ)]
            a_src = [avbs[j][:] for j in range(NJ)]
        else:
            x_src = [cats[j][:, 0, :] for j in range(NJ)]
            a_src = [cats[j][:, 1, :] for j in range(NJ)]

        for j in range(NJ):
            if j + 2 < NJ:
                w_tiles[j + 2] = load_pair_w(j + 2)
            wt_i, wt_f = w_tiles.pop(j)
            gt = gat_pool.tile([128, L], BF16, name="gt", tag="gt")
            tm = tmp_pool.tile([128, L], FP32, name="tm", tag="tm")
            st_i = sig_pool.tile([128, L], FP32, name="st", tag="st")
            st_f = sig_pool.tile([128, L], FP32, name="st", tag="st")
            for half, wt, st in ((0, wt_i, st_i), (1, wt_f, st_f)):
                ob = j + NJ * half
                pss = [
                    psum_pool.tile([128, TCW], FP32, name="ps", tag="ps")
                    for _ in range(NTC)
                ]
                if fp8:
                    for i in range(NJ):
                        for tcx in range(NTC):
                            nc.tensor.matmul(
                                pss[tcx][:],
                                wt[:, i, :, :],
                                cats[i][:, :, ts(tcx, TCW)],
                                start=(i == 0),
                                stop=(i == NJ - 1),
                                perf_mode=mybir.MatmulPerfMode.DoubleRow,
                            )
                else:
                    for i in range(NJ):
                        for s in range(2):
                            for tcx in range(NTC):
                                nc.tensor.matmul(
                                    pss[tcx][:],
                                    wt[:, i, s, :],
                                    cats[i][:, s, ts(tcx, TCW)],
                                    start=(i == 0 and s == 0),
                                    stop=(i == NJ - 1 and s == 1),
                                )
                for tcx in range(NTC):
                    nc.scalar.activation(
                        st[:, ts(tcx, TCW)],
                        pss[tcx][:],
                        mybir.ActivationFunctionType.Sigmoid,
                        bias=bias_sb[:, ob : ob + 1],
                        scale=ACT_SCALE if fp8 else 1.0,
                    )
            # Gate combine: i-gate product on the DVE, f-gate product on the
            # otherwise-idle Pool engine, sum (bf16 out) on the DVE. The last
            # pair combines and stores per token chunk to shorten the serial
            # tail after the final matmul.
            if j == NJ - 1:
                for tcx in range(NTC):
                    s = ts(tcx, TCW)
                    nc.vector.tensor_mul(tm[:, s], st_i[:, s], x_src[j][:, s])
                    nc.gpsimd.tensor_mul(st_f[:, s], st_f[:, s], a_src[j][:, s])
                    nc.vector.tensor_add(gt[:, s], tm[:, s], st_f[:, s])
                    nc.sync.dma_start(gatT[j][:, s], gt[:, s])
            else:
                nc.vector.tensor_mul(tm[:], st_i[:], x_src[j])
                nc.gpsimd.tensor_mul(st_f[:], st_f[:], a_src[j])
                nc.vector.tensor_add(gt[:], tm[:], st_f[:])
                nc.sync.dma_start(gatT[j], gt[:])


_CACHE: dict = {}


def build_nc(mm: str | None = None, reps: int | None = None):
    import os as _os

    if mm is None:
        mm = _os.environ.get("KMM", "fp8")
    if reps is None:
        reps = int(_os.environ.get("KREPS", "1"))
    key = ("nc", mm, reps, TCW)
    if key not in _CACHE:
        nc = bacc.Bacc(
            "TRN2",
            target_bir_lowering=False,
            debug=False,
            enable_asserts=True,
            num_devices=B,
        )
        with tile.TileContext(nc) as t:
            _tile_body(t, mm=mm, reps=reps)
        nc.compile()
        _CACHE[key] = nc
    return _CACHE[key]


def prep_shared(W_gate: np.ndarray, b_gate: np.ndarray, mm: str = "fp8"):
    Wf = np.asarray(W_gate, dtype=np.float32)
    fp8 = mm == "fp8"
    sxw, saw = (SXW, SAW) if fp8 else (1.0, 1.0)
    dt = ml_dtypes.float8_e4m3 if fp8 else ml_dtypes.bfloat16
    # wob[ob, p, j, s, o] = W'[128*ob + o, 128*(j + s*NJ) + p]
    Wq = Wf.reshape(NOB, 128, 2, NJ, 128)  # [ob, o, s, j, p]
    wob = np.empty((NOB, 128, NJ, 2, 128), dtype=dt)
    wob[:, :, :, 0, :] = (Wq[:, :, 0] * sxw).transpose(0, 3, 2, 1).astype(dt)
    wob[:, :, :, 1, :] = (Wq[:, :, 1] * saw).transpose(0, 3, 2, 1).astype(dt)
    invd = np.ascontiguousarray(
        np.broadcast_to(
            1.0 / np.arange(1, L + 1, dtype=np.float32)[None, :], (128, L)
        )
    )
    biash = np.ascontiguousarray(
        np.asarray(b_gate, dtype=np.float32).reshape(NOB, 128).T
    )
    return wob, invd, biash


def kernel(inputs: np.ndarray, W_gate: np.ndarray, b_gate: np.ndarray, **run_kwargs):
    import os as _os

    mm = _os.environ.get("KMM", "fp8")
    inputs = np.asarray(inputs, dtype=np.float32)
    assert inputs.shape == (B, L, D)

    wob, invd, biash = prep_shared(W_gate, b_gate, mm=mm)
    in_maps = []
    for c in range(B):
        xTb_c = inputs[c].T.reshape(NJ, 128, L).astype(ml_dtypes.bfloat16)
        in_maps.append({"xTb": xTb_c, "wob": wob, "invd": invd, "biash": biash})

    nc = build_nc(mm=mm)
    res = bass_utils.run_bass_kernel_spmd(
        nc, in_maps, core_ids=list(range(B)), **run_kwargs
    )

    gating = np.empty((B, L, D), dtype=np.float32)
    average = np.empty((B, L, D), dtype=np.float32)
    for c in range(B):
        gating[c] = res.results[c]["gatT"].reshape(D, L).T.astype(np.float32)
        average[c] = res.results[c]["avgT"].reshape(D, L).T.astype(np.float32)
    if run_kwargs:
        _CACHE["last_results"] = res
    return gating, average


# revision 14
# speedup vs baseline: 1.1223x; 1.1223x over previous
"""Trainium2 Bass kernel for nn_AverageAttention (B=8, L=2048, D=1024).

Math (per batch b):
    avg[t]  = cumsum(x, axis=t)[t] / (t+1)
    g       = concat([x, avg], -1) @ W_gate.T + b_gate        # (L, 2*D)
    out     = sigmoid(g[:, :D]) * x + sigmoid(g[:, D:]) * avg

Strategy: batch-parallel over 8 NeuronCores (one sequence per core), W_gate
replicated. On-chip layout is transposed (feature-on-partition,
token-on-free) so the cumulative sum is one DVE tensor_tensor_scan per
128-feature chunk. I/O is bf16 (x in, avg/gating out; fp32 upcast on host).

The gating matmul splits into an x half and an avg half. Modes (KMM):
  mix  (default): x half in bf16, avg half in fp8e4 DoubleRow (2 contraction
        rows per PE cell per cycle). The avg half carries little signal
        (avg ~ 1/sqrt(t)), so fp8 there costs almost no accuracy:
        rel_l2 ~2.6e-3 / relmax ~1.1e-2 at ~1.3x PE speedup.
  fp8:  both halves fp8 DoubleRow (~1.8x PE, rel_l2 ~1.2e-2).
  bf16: both halves bf16 (rel_l2 ~1.8e-3).
fp8 operands are scaled into the e4m3 normal range (TRN e4m3 max +-240):
every partial product carries a 32x scale (x-half W*32; avg-half cat*8,
W*4), undone by the activation scale (sigmoid(psum/32 + bias)) fused into
PSUM evacuation.
"""

from contextlib import ExitStack

import ml_dtypes
import numpy as np

import concourse.bass as bass
import concourse.bass_utils as bass_utils
import concourse.mybir as mybir
import concourse.tile as tile
from concourse import bacc
from concourse._compat import with_exitstack
from concourse.bass import ts

B, L, D = 8, 2048, 1024
NJ = D // 128         # 8 feature chunks (x/avg each)
NP = NJ // 2          # 4 DoubleRow chunk-pairs per half
NOB = 2 * D // 128    # 16 output-feature blocks of g
import os as _os_mod

TCW = int(_os_mod.environ.get("KTCW", "512"))  # matmul moving free-dim
NTC = L // TCW

FP32 = mybir.dt.float32
BF16 = mybir.dt.bfloat16
FP8 = mybir.dt.float8e4

# fp8 scale scheme (see module docstring)
SXW, SAW, SACAT = 32.0, 4.0, 8.0
DR = mybir.MatmulPerfMode.DoubleRow


def _mode(mm):
    # returns (x_dtype, avg_dtype, act_scale)
    return {
        "mix": (BF16, FP8, 1.0 / 32.0),
        "fp8": (FP8, FP8, 1.0 / 32.0),
        "bf16": (BF16, BF16, 1.0),
    }[mm]


@with_exitstack
def _tile_body(
    ctx: ExitStack,
    tc: tile.TileContext,
    mm: str = "mix",
    reps: int = 1,
):
    nc = tc.nc
    xdt, adt, act_scale = _mode(mm)

    # Weight tensors, one per half. Free layout per output block ob:
    #   fp8 half:  [128 contraction, pair, 2, 128 out]  (DoubleRow 3D AP)
    #   bf16 half: [128 contraction, chunk, 128 out]
    def w_shape(dt):
        return (NOB, 128, NP, 2, 128) if dt == FP8 else (NOB, 128, NJ, 128)

    xTb = nc.dram_tensor("xTb", (NJ, 128, L), BF16, kind="ExternalInput").ap()
    wobx = nc.dram_tensor("wobx", w_shape(xdt), xdt, kind="ExternalInput").ap()
    woba = nc.dram_tensor("woba", w_shape(adt), adt, kind="ExternalInput").ap()
    invd = nc.dram_tensor("invd", (128, L), BF16, kind="ExternalInput").ap()
    biash = nc.dram_tensor("biash", (128, NOB), FP32, kind="ExternalInput").ap()
    avgT = nc.dram_tensor("avgT", (NJ, 128, L), BF16, kind="ExternalOutput").ap()
    gatT = nc.dram_tensor("gatT", (NJ, 128, L), BF16, kind="ExternalOutput").ap()

    const_pool = ctx.enter_context(tc.tile_pool(name="const", bufs=1))
    x_pool = ctx.enter_context(tc.tile_pool(name="x", bufs=NJ))
    avg_pool = ctx.enter_context(tc.tile_pool(name="avg", bufs=NJ))
    cum_pool = ctx.enter_context(tc.tile_pool(name="cum", bufs=2))
    w_pool = ctx.enter_context(tc.tile_pool(name="w", bufs=6))
    sig_pool = ctx.enter_context(tc.tile_pool(name="sig", bufs=3))
    tmp_pool = ctx.enter_context(tc.tile_pool(name="tmp", bufs=2))
    gat_pool = ctx.enter_context(tc.tile_pool(name="gat", bufs=2))
    psum_pool = ctx.enter_context(
        tc.tile_pool(name="psum", bufs=max(1, 8 * 512 // TCW), space="PSUM")
    )
    if xdt == FP8 or adt == FP8:
        cat_pool = ctx.enter_context(tc.tile_pool(name="cat", bufs=2 * NP))

    invd_sb = const_pool.tile([128, L], BF16, tag="invd")
    bias_sb = const_pool.tile([128, NOB], FP32, tag="bias")
    if adt == FP8:
        invd8_sb = const_pool.tile([128, L], BF16, tag="invd8")

    # fp8 pair tiles: [:, s, :] = chunk 2k+s of that half
    xcat = (
        [cat_pool.tile([128, 2, L], FP8, tag="xc", name=f"xc{k}") for k in range(NP)]
        if xdt == FP8
        else None
    )
    acat = (
        [cat_pool.tile([128, 2, L], FP8, tag="ac", name=f"ac{k}") for k in range(NP)]
        if adt == FP8
        else None
    )

    def load_pair_w(j, chunks=1):
        # Pair j covers output blocks j (input gate) and NJ+j (forget gate).
        tiles = []
        for ob in (j, NJ + j):
            wx = w_pool.tile(
                [128, NP, 2, 128] if xdt == FP8 else [128, NJ, 128],
                xdt, name="wx", tag="wx",
            )
            wa = w_pool.tile(
                [128, NP, 2, 128] if adt == FP8 else [128, NJ, 128],
                adt, name="wa", tag="wa",
            )
            step = (NP if xdt == FP8 else NJ) // chunks
            stepa = (NP if adt == FP8 else NJ) // chunks
            for c in range(chunks):
                nc.sync.dma_start(
                    wx[:, c * step : (c + 1) * step], wobx[ob][:, c * step : (c + 1) * step]
                )
                nc.sync.dma_start(
                    wa[:, c * stepa : (c + 1) * stepa],
                    woba[ob][:, c * stepa : (c + 1) * stepa],
                )
            tiles.append((wx, wa))
        return tiles

    for _rep in range(reps):
        # Head ordering on the sync HWDGE ring (FIFO per ring): first x
        # chunk, then the first pair's W in chunks, so the first matmul
        # unblocks as early as possible. Constants ride the gpsimd SWDGE
        # ring so they never queue ahead of inputs.
        xts = [x_pool.tile([128, L], BF16, name="xt", tag="xt") for _ in range(NJ)]
        avbs = [avg_pool.tile([128, L], BF16, name="av", tag="av") for _ in range(NJ)]
        nc.sync.dma_start(xts[0][:], xTb[0])
        if _rep == 0:
            nc.gpsimd.dma_start(invd_sb[:], invd[:])
            nc.gpsimd.dma_start(bias_sb[:], biash[:])
            if adt == FP8:
                nc.gpsimd.tensor_scalar_mul(invd8_sb[:], invd_sb[:], SACAT)
        # Pair 0's weights load right after x0 (chunked, so the first
        # Ldweights waits only for a quarter tile); pair 1 loads after the x
        # stream — the PE touches it late enough, and x must win the ring.
        w_tiles = {0: load_pair_w(0, chunks=4)}

        # Phase 1: per feature chunk j — load x (bf16), full-width cumsum
        # scan on the DVE (fp32 state, bf16 out), avg = cum/(t+1) on Pool
        # (bf16, feeds the store + gate combine), avg*8 -> fp8 cat on the
        # DVE, x -> fp8 cat on ACT (fp8 x-half only).
        for j in range(NJ):
            if j > 0:
                nc.sync.dma_start(xts[j][:], xTb[j])
            ct = cum_pool.tile([128, L], BF16)
            nc.vector.tensor_tensor_scan(
                ct[:],
                xts[j][:],
                xts[j][:],
                0.0,
                mybir.AluOpType.add,
                mybir.AluOpType.bypass,
            )
            nc.gpsimd.tensor_mul(avbs[j][:], ct[:], invd_sb[:])
            if adt == FP8:
                nc.gpsimd.tensor_mul(acat[j // 2][:, j % 2, :], ct[:], invd8_sb[:])
            if xdt == FP8:
                nc.scalar.copy(xcat[j // 2][:, j % 2, :], xts[j][:])
            nc.scalar.dma_start(avgT[j], avbs[j][:])
        w_tiles[1] = load_pair_w(1)

        # Phase 2: gating matmul g^T = W @ cat^T per 128-row output block in
        # weight-stationary order (the NTC token chunks run as interleaved
        # PSUM groups so consecutive matmuls share stationary weights); the
        # x half accumulates first (ready early), then the avg half;
        # sigmoid(psum * act_scale + bias) fused into PSUM evacuation on the
        # scalar engine; elementwise gate combine per pair.
        def accum_half(pss, w, dt, pairs, chunks, first, last):
            if dt == FP8:
                for p in range(NP):
                    for tcx in range(NTC):
                        nc.tensor.matmul(
                            pss[tcx][:],
                            w[:, p, :, :],
                            pairs[p][:, :, ts(tcx, TCW)],
                            start=(first and p == 0),
                            stop=(last and p == NP - 1),
                            perf_mode=DR,
                        )
            else:
                for i in range(NJ):
                    for tcx in range(NTC):
                        nc.tensor.matmul(
                            pss[tcx][:],
                            w[:, i, :],
                            chunks[i][:, ts(tcx, TCW)],
                            start=(first and i == 0),
                            stop=(last and i == NJ - 1),
                        )

        for j in range(NJ):
            if j + 2 < NJ:
                w_tiles[j + 2] = load_pair_w(j + 2)
            pair_w = w_tiles.pop(j)
            gt = gat_pool.tile([128, L], BF16, name="gt", tag="gt")
            tm = tmp_pool.tile([128, L], BF16, name="tm", tag="tm")
            st_i = sig_pool.tile([128, L], BF16, name="st", tag="st")
            st_f = sig_pool.tile([128, L], BF16, name="st", tag="st")
            # Both halves' x accumulations run before either avg half: the
            # avg chunks come off the serial scan chain, so the two x halves
            # (~14us of PE work) cover the chain's tail. The two PSUM groups
            # stay open concurrently (8 banks total).
            pss2 = []
            for half in (0, 1):
                pss = [
                    psum_pool.tile([128, TCW], FP32, name="ps", tag="ps")
                    for _ in range(NTC)
                ]
                pss2.append(pss)
                accum_half(pss, pair_w[half][0], xdt, xcat, xts, True, False)
            for half in (0, 1):
                accum_half(pss2[half], pair_w[half][1], adt, acat, avbs, False, True)
            for half, st in ((0, st_i), (1, st_f)):
                ob = j + NJ * half
                for tcx in range(NTC):
                    nc.scalar.activation(
                        st[:, ts(tcx, TCW)],
                        pss2[half][tcx][:],
                        mybir.ActivationFunctionType.Sigmoid,
                        bias=bias_sb[:, ob : ob + 1],
                        scale=act_scale,
                    )
            # Gate combine: i-gate product on the DVE, f-gate product on the
            # otherwise-idle Pool engine, sum (bf16) on the DVE. The last
            # two pairs combine and store per token chunk to shorten the
            # serial tail after the final matmul.
            if j >= NJ - 2:
                for tcx in range(NTC):
                    s = ts(tcx, TCW)
                    nc.vector.tensor_mul(tm[:, s], st_i[:, s], xts[j][:, s])
                    nc.gpsimd.tensor_mul(st_f[:, s], st_f[:, s], avbs[j][:, s])
                    nc.vector.tensor_add(gt[:, s], tm[:, s], st_f[:, s])
                    nc.scalar.dma_start(gatT[j][:, s], gt[:, s])
            else:
                nc.vector.tensor_mul(tm[:], st_i[:], xts[j][:])
                nc.gpsimd.tensor_mul(st_f[:], st_f[:], avbs[j][:])
                nc.vector.tensor_add(gt[:], tm[:], st_f[:])
                nc.vector.dma_start(gatT[j], gt[:])


_CACHE: dict = {}


def build_nc(mm: str | None = None, reps: int | None = None):
    import os as _os

    if mm is None:
        mm = _os.environ.get("KMM", "mix")
    if reps is None:
        reps = int(_os.environ.get("KREPS", "1"))
    key = ("nc", mm, reps, TCW)
    if key not in _CACHE:
        nc = bacc.Bacc(
            "TRN2",
            target_bir_lowering=False,
            debug=False,
            enable_asserts=True,
            num_devices=B,
        )
        with tile.TileContext(nc) as t:
            _tile_body(t, mm=mm, reps=reps)
        nc.compile()
        _CACHE[key] = nc
    return _CACHE[key]


def prep_shared(W_gate: np.ndarray, b_gate: np.ndarray, mm: str = "mix"):
    Wf = np.asarray(W_gate, dtype=np.float32)
    xdt, adt, _ = _mode(mm)
    anyfp8 = FP8 in (xdt, adt)
    E4, BF = ml_dtypes.float8_e4m3, ml_dtypes.bfloat16

    def prep_half(Wh, dt, wscale):
        # Wh: (2048 out, 1024 in) for this half.
        Wq = (Wh * wscale).reshape(NOB, 128, NJ, 128)  # [ob, o, j, p]
        if dt == FP8:
            # [ob, p, pair, s, o], chunk = 2*pair + s
            return np.ascontiguousarray(
                Wq.transpose(0, 3, 2, 1).reshape(NOB, 128, NP, 2, 128)
            ).astype(E4)
        return np.ascontiguousarray(Wq.transpose(0, 3, 2, 1)).astype(BF)

    wobx = prep_half(Wf[:, :D], xdt, SXW if anyfp8 else 1.0)
    woba = prep_half(Wf[:, D:], adt, SAW if anyfp8 else 1.0)
    invd = np.ascontiguousarray(
        np.broadcast_to(
            1.0 / np.arange(1, L + 1, dtype=np.float32)[None, :], (128, L)
        )
    ).astype(BF)
    biash = np.ascontiguousarray(
        np.asarray(b_gate, dtype=np.float32).reshape(NOB, 128).T
    )
    return wobx, woba, invd, biash


def kernel(inputs: np.ndarray, W_gate: np.ndarray, b_gate: np.ndarray, **run_kwargs):
    import os as _os

    mm = _os.environ.get("KMM", "mix")
    inputs = np.asarray(inputs, dtype=np.float32)
    assert inputs.shape == (B, L, D)

    wobx, woba, invd, biash = prep_shared(W_gate, b_gate, mm=mm)
    in_maps = []
    for c in range(B):
        xTb_c = inputs[c].T.reshape(NJ, 128, L).astype(ml_dtypes.bfloat16)
        in_maps.append(
            {"xTb": xTb_c, "wobx": wobx, "woba": woba, "invd": invd, "biash": biash}
        )

    nc = build_nc(mm=mm)
    res = bass_utils.run_bass_kernel_spmd(
        nc, in_maps, core_ids=list(range(B)), **run_kwargs
    )

    gating = np.empty((B, L, D), dtype=np.float32)
    average = np.empty((B, L, D), dtype=np.float32)
    for c in range(B):
        gating[c] = res.results[c]["gatT"].reshape(D, L).T.astype(np.float32)
        average[c] = res.results[c]["avgT"].reshape(D, L).T.astype(np.float32)
    if run_kwargs:
        _CACHE["last_results"] = res
    return gating, average


# revision 17
# speedup vs baseline: 1.1703x; 1.0427x over previous
"""Trainium2 Bass kernel for nn_AverageAttention (B=8, L=2048, D=1024).

Math (per batch b):
    avg[t]  = cumsum(x, axis=t)[t] / (t+1)
    g       = concat([x, avg], -1) @ W_gate.T + b_gate        # (L, 2*D)
    out     = sigmoid(g[:, :D]) * x + sigmoid(g[:, D:]) * avg

Strategy: batch-parallel over 8 NeuronCores (one sequence per core), W_gate
replicated. On-chip layout is transposed (feature-on-partition,
token-on-free) so the cumulative sum is one DVE tensor_tensor_scan per
128-feature chunk. I/O is bf16 (x in, avg/gating out; fp32 upcast on host).

The gating matmul splits into an x half and an avg half. Modes (KMM):
  mix  (default): x half in bf16, avg half in fp8e4 DoubleRow (2 contraction
        rows per PE cell per cycle). The avg half carries little signal
        (avg ~ 1/sqrt(t)), so fp8 there costs almost no accuracy:
        rel_l2 ~2.6e-3 / relmax ~1.1e-2 at ~1.3x PE speedup.
  fp8:  both halves fp8 DoubleRow (~1.8x PE, rel_l2 ~1.2e-2).
  bf16: both halves bf16 (rel_l2 ~1.8e-3).
fp8 operands are scaled into the e4m3 normal range (TRN e4m3 max +-240):
every partial product carries a 32x scale (x-half W*32; avg-half cat*8,
W*4), undone by the activation scale (sigmoid(psum/32 + bias)) fused into
PSUM evacuation.
"""

from contextlib import ExitStack

import ml_dtypes
import numpy as np

import concourse.bass as bass
import concourse.bass_utils as bass_utils
import concourse.mybir as mybir
import concourse.tile as tile
from concourse import bacc
from concourse._compat import with_exitstack
from concourse.bass import ts

B, L, D = 8, 2048, 1024
NJ = D // 128         # 8 feature chunks (x/avg each)
NP = NJ // 2          # 4 DoubleRow chunk-pairs per half
NOB = 2 * D // 128    # 16 output-feature blocks of g
import os as _os_mod

TCW = int(_os_mod.environ.get("KTCW", "512"))  # matmul moving free-dim
NTC = L // TCW

FP32 = mybir.dt.float32
BF16 = mybir.dt.bfloat16
FP8 = mybir.dt.float8e4

# fp8 scale scheme (see module docstring)
SXW, SAW, SACAT = 32.0, 4.0, 8.0
DR = mybir.MatmulPerfMode.DoubleRow


def _mode(mm):
    # returns (x_dtype, avg_dtype, act_scale)
    return {
        "mix": (BF16, FP8, 1.0 / 32.0),
        "fp8": (FP8, FP8, 1.0 / 32.0),
        "bf16": (BF16, BF16, 1.0),
    }[mm]


@with_exitstack
def _tile_body(
    ctx: ExitStack,
    tc: tile.TileContext,
    mm: str = "mix",
    reps: int = 1,
):
    nc = tc.nc
    xdt, adt, act_scale = _mode(mm)

    # Weight tensors, one per half. Free layout per output block ob:
    #   fp8 half:  [128 contraction, pair, 2, 128 out]  (DoubleRow 3D AP)
    #   bf16 half: [128 contraction, chunk, 128 out]
    def w_shape(dt):
        return (NOB, 128, NP, 2, 128) if dt == FP8 else (NOB, 128, NJ, 128)

    xTb = nc.dram_tensor("xTb", (NJ, 128, L), BF16, kind="ExternalInput").ap()
    wobx = nc.dram_tensor("wobx", w_shape(xdt), xdt, kind="ExternalInput").ap()
    woba = nc.dram_tensor("woba", w_shape(adt), adt, kind="ExternalInput").ap()
    invd = nc.dram_tensor("invd", (128, L), BF16, kind="ExternalInput").ap()
    biash = nc.dram_tensor("biash", (128, NOB), FP32, kind="ExternalInput").ap()
    avgT = nc.dram_tensor("avgT", (NJ, 128, L), BF16, kind="ExternalOutput").ap()
    gatT = nc.dram_tensor("gatT", (NJ, 128, L), BF16, kind="ExternalOutput").ap()

    const_pool = ctx.enter_context(tc.tile_pool(name="const", bufs=1))
    x_pool = ctx.enter_context(tc.tile_pool(name="x", bufs=NJ))
    avg_pool = ctx.enter_context(tc.tile_pool(name="avg", bufs=NJ))
    cum_pool = ctx.enter_context(tc.tile_pool(name="cum", bufs=2))
    w_pool = ctx.enter_context(tc.tile_pool(name="w", bufs=6))
    sig_pool = ctx.enter_context(tc.tile_pool(name="sig", bufs=3))
    tmp_pool = ctx.enter_context(tc.tile_pool(name="tmp", bufs=2))
    gat_pool = ctx.enter_context(tc.tile_pool(name="gat", bufs=2))
    psum_pool = ctx.enter_context(
        tc.tile_pool(name="psum", bufs=max(1, 8 * 512 // TCW), space="PSUM")
    )
    if xdt == FP8 or adt == FP8:
        cat_pool = ctx.enter_context(tc.tile_pool(name="cat", bufs=2 * NP))

    invd_sb = const_pool.tile([128, L], BF16, tag="invd")
    bias_sb = const_pool.tile([128, NOB], FP32, tag="bias")
    if adt == FP8:
        invd8_sb = const_pool.tile([128, L], BF16, tag="invd8")

    # fp8 pair tiles: [:, s, :] = chunk 2k+s of that half
    xcat = (
        [cat_pool.tile([128, 2, L], FP8, tag="xc", name=f"xc{k}") for k in range(NP)]
        if xdt == FP8
        else None
    )
    acat = (
        [cat_pool.tile([128, 2, L], FP8, tag="ac", name=f"ac{k}") for k in range(NP)]
        if adt == FP8
        else None
    )

    def load_pair_w(j, chunks=1):
        # Pair j covers output blocks j (input gate) and NJ+j (forget gate).
        tiles = []
        for ob in (j, NJ + j):
            wx = w_pool.tile(
                [128, NP, 2, 128] if xdt == FP8 else [128, NJ, 128],
                xdt, name="wx", tag="wx",
            )
            wa = w_pool.tile(
                [128, NP, 2, 128] if adt == FP8 else [128, NJ, 128],
                adt, name="wa", tag="wa",
            )
            step = (NP if xdt == FP8 else NJ) // chunks
            stepa = (NP if adt == FP8 else NJ) // chunks
            for c in range(chunks):
                nc.sync.dma_start(
                    wx[:, c * step : (c + 1) * step], wobx[ob][:, c * step : (c + 1) * step]
                )
                nc.sync.dma_start(
                    wa[:, c * stepa : (c + 1) * stepa],
                    woba[ob][:, c * stepa : (c + 1) * stepa],
                )
            tiles.append((wx, wa))
        return tiles

    for _rep in range(reps):
        # Head ordering on the sync HWDGE ring (FIFO per ring): first x
        # chunk, then the first pair's W in chunks, so the first matmul
        # unblocks as early as possible. Constants ride the gpsimd SWDGE
        # ring so they never queue ahead of inputs.
        xts = [x_pool.tile([128, L], BF16, name="xt", tag="xt") for _ in range(NJ)]
        avbs = [avg_pool.tile([128, L], BF16, name="av", tag="av") for _ in range(NJ)]
        nc.sync.dma_start(xts[0][:], xTb[0])
        if _rep == 0:
            nc.gpsimd.dma_start(invd_sb[:], invd[:])
            nc.gpsimd.dma_start(bias_sb[:], biash[:])
            if adt == FP8:
                nc.gpsimd.tensor_scalar_mul(invd8_sb[:], invd_sb[:], SACAT)
        # Pair 0's weights load right after x0 (chunked, so the first
        # Ldweights waits only for a quarter tile); pair 1 loads after the x
        # stream — the PE touches it late enough, and x must win the ring.
        w_tiles = {0: load_pair_w(0, chunks=4)}

        # Phase 1: per feature chunk j — load x (bf16), full-width cumsum
        # scan on the DVE (fp32 state, bf16 out), avg = cum/(t+1) on Pool
        # (bf16, feeds the store + gate combine), avg*8 -> fp8 cat on the
        # DVE, x -> fp8 cat on ACT (fp8 x-half only).
        for j in range(NJ):
            if j > 0:
                nc.sync.dma_start(xts[j][:], xTb[j])
            ct = cum_pool.tile([128, L], BF16)
            nc.vector.tensor_tensor_scan(
                ct[:],
                xts[j][:],
                xts[j][:],
                0.0,
                mybir.AluOpType.add,
                mybir.AluOpType.bypass,
            )
            nc.gpsimd.tensor_mul(avbs[j][:], ct[:], invd_sb[:])
            if adt == FP8:
                nc.gpsimd.tensor_mul(acat[j // 2][:, j % 2, :], ct[:], invd8_sb[:])
            if xdt == FP8:
                nc.scalar.copy(xcat[j // 2][:, j % 2, :], xts[j][:])
            nc.scalar.dma_start(avgT[j], avbs[j][:])
        w_tiles[1] = load_pair_w(1)

        # Phase 2: gating matmul g^T = W @ cat^T per 128-row output block in
        # weight-stationary order (the NTC token chunks run as interleaved
        # PSUM groups so consecutive matmuls share stationary weights); the
        # x half accumulates first (ready early), then the avg half;
        # sigmoid(psum * act_scale + bias) fused into PSUM evacuation on the
        # scalar engine; elementwise gate combine per pair.
        def accum_half(pss, w, dt, pairs, chunks, first, last):
            if dt == FP8:
                for p in range(NP):
                    for tcx in range(NTC):
                        nc.tensor.matmul(
                            pss[tcx][:],
                            w[:, p, :, :],
                            pairs[p][:, :, ts(tcx, TCW)],
                            start=(first and p == 0),
                            stop=(last and p == NP - 1),
                            perf_mode=DR,
                        )
            else:
                for i in range(NJ):
                    for tcx in range(NTC):
                        nc.tensor.matmul(
                            pss[tcx][:],
                            w[:, i, :],
                            chunks[i][:, ts(tcx, TCW)],
                            start=(first and i == 0),
                            stop=(last and i == NJ - 1),
                        )

        for j in range(NJ):
            if j + 2 < NJ:
                w_tiles[j + 2] = load_pair_w(j + 2)
            pair_w = w_tiles.pop(j)
            gt = gat_pool.tile([128, L], BF16, name="gt", tag="gt")
            tm = tmp_pool.tile([128, L], BF16, name="tm", tag="tm")
            st_i = sig_pool.tile([128, L], BF16, name="st", tag="st")
            st_f = sig_pool.tile([128, L], BF16, name="st", tag="st")
            # Both halves' x accumulations run before either avg half: the
            # avg chunks come off the serial scan chain, so the two x halves
            # (~14us of PE work) cover the chain's tail. The two PSUM groups
            # stay open concurrently (8 banks total).
            pss2 = []
            for half in (0, 1):
                pss = [
                    psum_pool.tile([128, TCW], FP32, name="ps", tag="ps")
                    for _ in range(NTC)
                ]
                pss2.append(pss)
                accum_half(pss, pair_w[half][0], xdt, xcat, xts, True, False)
            for half in (0, 1):
                accum_half(pss2[half], pair_w[half][1], adt, acat, avbs, False, True)
            # Sigmoids interleaved i/f per token chunk: the combine for a
            # chunk needs both gates, and PSUM banks recycle sooner.
            for tcx in range(NTC):
                for half, st in ((0, st_i), (1, st_f)):
                    nc.scalar.activation(
                        st[:, ts(tcx, TCW)],
                        pss2[half][tcx][:],
                        mybir.ActivationFunctionType.Sigmoid,
                        bias=bias_sb[:, j + NJ * half : j + NJ * half + 1],
                        scale=act_scale,
                    )
            # Gate combine: i-gate product on the DVE, f-gate product on the
            # otherwise-idle Pool engine, sum (bf16) on the DVE. The last
            # two pairs combine and store per token chunk to shorten the
            # serial tail after the final matmul.
            if j >= NJ - 2:
                for tcx in range(NTC):
                    s = ts(tcx, TCW)
                    nc.vector.tensor_mul(tm[:, s], st_i[:, s], xts[j][:, s])
                    nc.gpsimd.tensor_mul(st_f[:, s], st_f[:, s], avbs[j][:, s])
                    nc.vector.tensor_add(gt[:, s], tm[:, s], st_f[:, s])
                    nc.scalar.dma_start(gatT[j][:, s], gt[:, s])
            else:
                nc.vector.tensor_mul(tm[:], st_i[:], xts[j][:])
                nc.gpsimd.tensor_mul(st_f[:], st_f[:], avbs[j][:])
                nc.vector.tensor_add(gt[:], tm[:], st_f[:])
                nc.gpsimd.dma_start(gatT[j], gt[:])


_CACHE: dict = {}


def build_nc(mm: str | None = None, reps: int | None = None):
    import os as _os

    if mm is None:
        mm = _os.environ.get("KMM", "mix")
    if reps is None:
        reps = int(_os.environ.get("KREPS", "1"))
    key = ("nc", mm, reps, TCW)
    if key not in _CACHE:
        nc = bacc.Bacc(
            "TRN2",
            target_bir_lowering=False,
            debug=False,
            enable_asserts=True,
            num_devices=B,
        )
        with tile.TileContext(nc) as t:
            _tile_body(t, mm=mm, reps=reps)
        nc.compile()
        _CACHE[key] = nc
    return _CACHE[key]


def prep_shared(W_gate: np.ndarray, b_gate: np.ndarray, mm: str = "mix"):
    Wf = np.asarray(W_gate, dtype=np.float32)
    xdt, adt, _ = _mode(mm)
    anyfp8 = FP8 in (xdt, adt)
    E4, BF = ml_dtypes.float8_e4m3, ml_dtypes.bfloat16

    def prep_half(Wh, dt, wscale):
        # Wh: (2048 out, 1024 in) for this half.
        Wq = (Wh * wscale).reshape(NOB, 128, NJ, 128)  # [ob, o, j, p]
        if dt == FP8:
            # [ob, p, pair, s, o], chunk = 2*pair + s
            return np.ascontiguousarray(
                Wq.transpose(0, 3, 2, 1).reshape(NOB, 128, NP, 2, 128)
            ).astype(E4)
        return np.ascontiguousarray(Wq.transpose(0, 3, 2, 1)).astype(BF)

    wobx = prep_half(Wf[:, :D], xdt, SXW if anyfp8 else 1.0)
    woba = prep_half(Wf[:, D:], adt, SAW if anyfp8 else 1.0)
    invd = np.ascontiguousarray(
        np.broadcast_to(
            1.0 / np.arange(1, L + 1, dtype=np.float32)[None, :], (128, L)
        )
    ).astype(BF)
    biash = np.ascontiguousarray(
        np.asarray(b_gate, dtype=np.float32).reshape(NOB, 128).T
    )
    return wobx, woba, invd, biash


def kernel(inputs: np.ndarray, W_gate: np.ndarray, b_gate: np.ndarray, **run_kwargs):
    import os as _os

    mm = _os.environ.get("KMM", "mix")
    inputs = np.asarray(inputs, dtype=np.float32)
    assert inputs.shape == (B, L, D)

    wobx, woba, invd, biash = prep_shared(W_gate, b_gate, mm=mm)
    in_maps = []
    for c in range(B):
        xTb_c = inputs[c].T.reshape(NJ, 128, L).astype(ml_dtypes.bfloat16)
        in_maps.append(
            {"xTb": xTb_c, "wobx": wobx, "woba": woba, "invd": invd, "biash": biash}
        )

    nc = build_nc(mm=mm)
    res = bass_utils.run_bass_kernel_spmd(
        nc, in_maps, core_ids=list(range(B)), **run_kwargs
    )

    gating = np.empty((B, L, D), dtype=np.float32)
    average = np.empty((B, L, D), dtype=np.float32)
    for c in range(B):
        gating[c] = res.results[c]["gatT"].reshape(D, L).T.astype(np.float32)
        average[c] = res.results[c]["avgT"].reshape(D, L).T.astype(np.float32)
    if run_kwargs:
        _CACHE["last_results"] = res
    return gating, average
